# revision 54
# baseline (speedup 1.0000x reference)
"""Trainium2 Bass kernel for MinimalLBS (B=32, T=128, N=2048, J=52, Jb=21, L=16).

Strategy: data-parallel over B across 8 NeuronCores (4 samples per core).

Key algebraic restructure vs the naive path ("Psi-trick"):
  sens[n,i,t] = sum_{k,j} w[n,k] * A_aug[k,i,j,t] * vh[n,j,t]
with vh = vth + dh, vth = (v_template, 1), dh = (delta, 0),
delta = blend_shape + pose_offsets (small, ~0.05 sigma):

  S1[n,i,t] = sum_{(k,j)} Psi[n,(k,j)] * A_aug[(k,j),i,t]   (Psi = w (x) vth,
              t-independent -> host-computed, one bf16 matmul K=209; absorbs
              v_template, translation and the homogeneous j=3 column)
  S2[n,i,t] = sum_{j<3} ts[n,i,j,t] * delta[n,j,t]          (small correction;
              all inputs fp8 DoubleRow matmuls at 0.5 cycles/row)
  sens = S1 + S2

Per chunk of 128 vertices (per sample):
  PE : S1    = psit^T @ arb  2x bf16 matmuls (K=209) -> own 1-bank PSUM slot
       delta = pd8^T @ pft8  3x fp8-DR matmuls (K=206) -> spare tail row of
               each ts9 bank
       ts9   = wt8^T @ ar8   3x fp8-DR matmuls (K=52, j<3 only) -> 3-bank slot
  ACT: one copy evacuates delta+ts9 to bf16 SBUF; its scale slot undoes the
       x64 fp8 range prescale (this 1536-elem copy paces the pipeline)
  DVE: pm = ts9*delta (2x bf16), sb = pm2 + S1(PSUM), sens = sa + sb
  Pool: sa = pm0 + pm1
  Out-DMAs go in 4-chunk groups, emitted ~4 chunks late (and the last group
  after the next sample's input loads) so the SP queue never parks on a long
  semaphore wait and input prefetch is never blocked.
"""

import sys

sys.path.insert(0, "/opt/trn_rl_repo")

import math

import ml_dtypes
import numpy as np

import concourse.bacc as bacc
import concourse.bass as bass
import concourse.mybir as mybir
import concourse.tile as tile
from concourse import bass_utils, masks

F32 = mybir.dt.float32
BF16 = mybir.dt.bfloat16
F8 = mybir.dt.float8e4
NPBF16 = ml_dtypes.bfloat16
NPF8 = ml_dtypes.float8_e4m3fn
DR = mybir.MatmulPerfMode.DoubleRow

B, T, N, JB, J, L = 32, 128, 2048, 21, 52, 16
NCORES = 8
NB = B // NCORES          # samples per core
PF = JB * 9               # 189 pose-feature dims
NCH = N // 128            # n-chunks per sample
KD = PF + L + 1           # 206 logical K for the delta matmul (pad row last)
KDH = KD // 2             # 103
KS = J                    # 52 logical K for the ts9 matmul
KSH = KS // 2             # 26
KT = J * 4 + 1            # 209 logical K for the S1 matmul
KT0 = 128
KT1 = KT - 128            # 81
PD_SCALE = 64.0           # fp8 range prescale for posedirs/shapedirs

_CACHED = {}


def _build_nc():
    nc = bacc.Bacc("TRN2", target_bir_lowering=False, debug=False)

    pose_d = nc.dram_tensor("pose", [T, NB, JB, 3], F32, kind="ExternalInput")
    pd8_d = nc.dram_tensor("pd8", [NB, 3, KDH, 2, N], F8, kind="ExternalInput")
    beta8_d = nc.dram_tensor("beta8", [NB, L, T], F8, kind="ExternalInput")
    wt8_d = nc.dram_tensor("wt8", [NB, KSH, 2, N], F8, kind="ExternalInput")
    ar8_d = nc.dram_tensor("ar8", [NB, KSH, 2, 3, 3, T], F8, kind="ExternalInput")
    psit_d = nc.dram_tensor("psit", [NB, KT, N], BF16, kind="ExternalInput")
    arb_d = nc.dram_tensor("arb", [NB, KT, 3, T], BF16, kind="ExternalInput")
    out_d = nc.dram_tensor("out", [NB, 128, NCH, 3, T], BF16, kind="ExternalOutput")

    with tile.TileContext(nc) as tc:
        with (
            tc.tile_pool(name="const", bufs=1) as p_const,
            tc.tile_pool(name="rod", bufs=1) as p_rod,
            tc.tile_pool(name="big", bufs=2) as p_big,
            tc.tile_pool(name="small", bufs=2) as p_small,
            tc.tile_pool(name="mv", bufs=16) as p_mv,
            tc.tile_pool(name="ps", bufs=2, space="PSUM") as ps_ps,
            tc.tile_pool(name="s1p", bufs=2, space="PSUM") as ps_s1,
        ):
            ident = p_const.tile([128, 128], BF16)
            masks.make_identity(nc, ident[:])

            # ---- Rodrigues for all NB samples at once, ACT-free: with
            # y = |aa|^2, sinc(y) = sin(x)/x and g(y) = (1-cos x)/x^2 are
            # smooth in y, so (R - I) needs no sqrt/sin/reciprocal:
            #   diag_i   = g*(po_i^2 - y)
            #   offdiag  = g*po_a*po_b -+ sinc*po_c
            # Degree-3 series: error < 2e-5 over this pose range — far below
            # the fp8 quantization that follows.  Keeps the ACT table free
            # for the Copy-only steady state (no Sin/Sqrt table swaps).
            po = p_rod.tile([T, NB, JB, 3], F32)
            nc.sync.dma_start(po[:], pose_d[:])
            sq = p_rod.tile([T, NB, JB, 3], F32)
            nc.vector.tensor_tensor(sq[:], po[:], po[:], mybir.AluOpType.mult)
            a2 = p_rod.tile([T, NB, JB], F32)
            nc.vector.tensor_tensor(
                a2[:], sq[:, :, :, 0], sq[:, :, :, 1], mybir.AluOpType.add
            )
            y = p_rod.tile([T, NB, JB], F32)
            nc.vector.tensor_tensor(y[:], a2[:], sq[:, :, :, 2], mybir.AluOpType.add)

            def horner(coeffs, tag):
                # (((c3*y + c2)*y + c1)*y + c0  via alternating ts/tt ops
                h = p_rod.tile([T, NB, JB], F32, tag=f"{tag}_m")
                nc.vector.tensor_scalar(
                    h[:], y[:], coeffs[0], coeffs[1],
                    mybir.AluOpType.mult, mybir.AluOpType.add,
                )
                for ci, c in enumerate(coeffs[2:]):
                    hy = p_rod.tile([T, NB, JB], F32, tag=f"{tag}_h{ci}")
                    nc.vector.tensor_tensor(
                        hy[:], h[:], y[:], mybir.AluOpType.mult
                    )
                    h = p_rod.tile([T, NB, JB], F32, tag=f"{tag}_a{ci}")
                    nc.vector.tensor_scalar_add(h[:], hy[:], c)
                return h

            # 2/3-term series suffice: error ~5e-4 relative, far under the
            # fp8 quantization (1.7% rms) of the delta path these feed
            sinc = horner([1.0 / 120, -1.0 / 6, 1.0], "sinc")
            g = horner([1.0 / 720, -1.0 / 24, 0.5], "g")

            pf = p_rod.tile([T, NB, JB, 9], BF16)

            def poc(i):
                return po[:, :, :, i]

            prods = {}
            for (a, b2), nm in [((0, 1), "xy"), ((0, 2), "xz"), ((1, 2), "yz")]:
                t_ = p_rod.tile([T, NB, JB], F32, tag=f"pr_{nm}")
                nc.gpsimd.tensor_tensor(t_[:], poc(a), poc(b2), mybir.AluOpType.mult)
                prods[nm] = t_
            qs = {}
            for i, nm in [(0, "qx"), (1, "qy"), (2, "qz")]:
                t_ = p_rod.tile([T, NB, JB], F32, tag=f"q_{nm}")
                nc.vector.tensor_tensor(t_[:], sinc[:], poc(i), mybir.AluOpType.mult)
                qs[nm] = t_
            os_ = {}
            for nm in ["xy", "xz", "yz"]:
                t_ = p_rod.tile([T, NB, JB], F32, tag=f"o_{nm}")
                nc.vector.tensor_tensor(
                    t_[:], g[:], prods[nm][:], mybir.AluOpType.mult
                )
                os_[nm] = t_
            # diagonal: pf_di = g*(sq_i - y)
            d3 = p_rod.tile([T, NB, JB, 3], F32)
            nc.vector.tensor_tensor(
                d3[:], sq[:], y[:].unsqueeze(3).broadcast_to((T, NB, JB, 3)),
                mybir.AluOpType.subtract,
            )
            for di, i in [(0, 0), (4, 1), (8, 2)]:
                nc.vector.tensor_tensor(
                    pf[:, :, :, di], g[:], d3[:, :, :, i], mybir.AluOpType.mult
                )
            for ei, (e, o_nm, q_nm, op) in enumerate([
                (1, "xy", "qz", mybir.AluOpType.subtract),
                (3, "xy", "qz", mybir.AluOpType.add),
                (2, "xz", "qy", mybir.AluOpType.add),
                (6, "xz", "qy", mybir.AluOpType.subtract),
                (5, "yz", "qx", mybir.AluOpType.subtract),
                (7, "yz", "qx", mybir.AluOpType.add),
            ]):
                eng = nc.vector if ei % 2 == 0 else nc.gpsimd
                eng.tensor_tensor(
                    pf[:, :, :, e], os_[o_nm][:], qs[q_nm][:], op
                )

            def load_sample_inputs(nb, split=False):
                # For sample 0 the n-major tensors are split: the first 384
                # vertex columns (3 chunks' worth) land in ~1.5us so the
                # chunk pipeline starts while the bulk streams in behind.
                pd8_s = p_big.tile([KDH, 3, 2, N], F8, tag="pd8")
                pd8_src = pd8_d[nb].rearrange("c k g n -> k c g n")
                psit_a = p_small.tile([KT0, N], BF16, tag="psit_a")
                psit_b = p_small.tile([KT1, N], BF16, tag="psit_b")
                wt8_s = p_small.tile([KSH, 2, N], F8, tag="wt8")
                arb_a = p_small.tile([KT0, 3, T], BF16, tag="arb_a")
                arb_b = p_small.tile([KT1, 3, T], BF16, tag="arb_b")
                ar8_s = p_small.tile([KSH, 2, 3, 3, T], F8, tag="ar8")
                if split:
                    n1 = 384
                    for c in range(3):
                        nc.sync.dma_start(
                            pd8_s[:, c, :, 0:n1], pd8_src[:, c, :, 0:n1]
                        )
                else:
                    n1 = N
                    nc.sync.dma_start(pd8_s[:], pd8_src)
                nc.sync.dma_start(wt8_s[:, :, 0:n1], wt8_d[nb][:, :, 0:n1])
                nc.sync.dma_start(ar8_s[:], ar8_d[nb])
                nc.sync.dma_start(psit_a[:, 0:n1], psit_d[nb, 0:KT0, 0:n1])
                nc.sync.dma_start(arb_a[:], arb_d[nb, 0:KT0])
                nc.sync.dma_start(psit_b[:, 0:n1], psit_d[nb, KT0:KT, 0:n1])
                nc.sync.dma_start(arb_b[:], arb_d[nb, KT0:KT])
                if split:
                    for c in range(3):
                        nc.sync.dma_start(
                            pd8_s[:, c, :, n1:N], pd8_src[:, c, :, n1:N]
                        )
                    nc.sync.dma_start(psit_a[:, n1:N], psit_d[nb, 0:KT0, n1:N])
                    nc.sync.dma_start(wt8_s[:, :, n1:N], wt8_d[nb][:, :, n1:N])
                    nc.sync.dma_start(psit_b[:, n1:N], psit_d[nb, KT0:KT, n1:N])
                return pd8_s, wt8_s, ar8_s, psit_a, psit_b, arb_a, arb_b

            # beta loads ride the Pool software-DGE path (tiny transfers,
            # no HWDGE queue slot), then sample-0's bulk inputs
            pft8s = []
            for nb in range(NB):
                pft8 = p_const.tile([KDH, 2, T], F8, tag=f"pft8_{nb}")
                nc.vector.memset(pft8[:], 0.0)
                nc.gpsimd.dma_start(pft8[PF - KDH : PF - KDH + L, 1, :], beta8_d[nb])
                pft8s.append(pft8)
            s0_inputs = load_sample_inputs(0)

            # ---- pft8 transposes for ALL samples up front (PSUM slots and
            # PE are idle here; doing it per-sample stalled each boundary on
            # the s1p ring).  pft8 [103, 2, T]: fp8 DoubleRow K-groups of the
            # delta contraction: group0 = pf rows 0..102, group1 = pf rows
            # 103..188 ++ betas (16, DMA'd above) ++ zero pad row.  Sample
            # 0's copies run on ACT (ahead of the evac stream); later
            # samples' on DVE so they don't delay the first evacuations.
            # pack 4 transpose outputs (256B each) per 1-bank s1p slot so
            # the transposes never wait on the ring
            tp_a = ps_s1.tile([KDH, 8, T], BF16, tag="s1p")
            tp_b = ps_s1.tile([KDH, 8, T], BF16, tag="s1p")
            tps = [tp_a, tp_b]
            # dummy transposes into spare tp columns lift the PE out of its
            # cold p-state before the real transposes arrive
            for wi in (2, 3, 6):
                nc.tensor.transpose(
                    tp_a[:, wi, :], ident[:, 0:KDH], ident[:]
                )

            def emit_pft8(nb):
                pf_nb = pf[:, nb].rearrange("t j e -> t (j e)")
                tp = tps[nb // 2]
                o = 4 * (nb % 2)
                nc.tensor.transpose(tp[:, o, :], pf_nb[:, 0:KDH], ident[:])
                nc.tensor.transpose(
                    tp[0 : PF - KDH, o + 1, :], pf_nb[:, KDH:PF], ident[:]
                )
                pft8 = pft8s[nb]
                cp = nc.scalar.copy if nb == 0 else nc.vector.tensor_copy
                cp(pft8[:, 0, :], tp[:, o, :])
                cp(pft8[0 : PF - KDH, 1, :], tp[0 : PF - KDH, o + 1, :])

            # only sample 0's pft8 gates the first chunk; samples 1-3 are
            # emitted after chunk 0 so their transposes ride PE slack instead
            # of sitting ahead of chunk 0 in the PE stream
            emit_pft8(0)

            # ---- per-sample pipeline
            prev_out = None
            for nb in range(NB):
                pft8 = pft8s[nb]
                if nb == 0:
                    (pd8_s, wt8_s, ar8_s, psit_a, psit_b, arb_a,
                     arb_b) = s0_inputs
                else:
                    (pd8_s, wt8_s, ar8_s, psit_a, psit_b, arb_a,
                     arb_b) = load_sample_inputs(nb)

                # whole-sample output staging; DMA'd out in 4-chunk groups.
                # The previous sample's last half is emitted here, after this
                # sample's input loads, so its wait doesn't block them.
                if prev_out is not None:
                    nc.sync.dma_start(
                        out_d[prev_out[0], :, 8:16], prev_out[1][:, 8:16]
                    )
                outs = p_small.tile([128, NCH, 3, T], BF16, tag="outs")

                for nch in range(NCH):
                    n0 = nch * 128
                    # 3-bank slot: bank i holds ts9_i (rows 0:3) and delta_c=i
                    # in the spare tail row 3 — all matmul outs bank-local.
                    # delta/ts9 first in the PE stream: the evac only needs
                    # these, so a late psit/arb load can't stall it.
                    ps = ps_ps.tile([128, 3, 4, T], F32, tag="ps")
                    for c in range(3):
                        nc.tensor.matmul(
                            ps[:, c, 3, :],
                            pd8_s[:, c, :, n0 : n0 + 128],
                            pft8[:],
                            start=True, stop=True, perf_mode=DR,
                        )
                    for i in range(3):
                        nc.tensor.matmul(
                            ps[:, i, 0:3, :],
                            wt8_s[:, :, n0 : n0 + 128],
                            ar8_s[:, :, i],
                            start=True, stop=True, perf_mode=DR,
                        )
                    # S1 in its own 1-bank slot, only needed by sb later
                    s1p = ps_s1.tile([128, 3, T], F32, tag="s1p")
                    nc.tensor.matmul(
                        s1p[:], psit_a[:, n0 : n0 + 128], arb_a[:],
                        start=True, stop=False,
                    )
                    nc.tensor.matmul(
                        s1p[:], psit_b[:, n0 : n0 + 128], arb_b[:],
                        start=False, stop=True,
                    )
                    # single ACT copy evacuates delta + ts9 together; its
                    # scale slot undoes the x64 fp8 prescale on both factors
                    dtsb = p_mv.tile([128, 3, 4, T], BF16, tag="dtsb")
                    nc.scalar.mul(dtsb[:], ps[:], 1.0 / PD_SCALE)

                    dbb = (
                        dtsb[:, :, 3, :].unsqueeze(1)
                        .broadcast_to((128, 3, 3, T))
                    )
                    pm = p_mv.tile([128, 3, 3, T], BF16, tag="pm")
                    sa = p_mv.tile([128, 3, T], BF16, tag="sa")
                    sb = p_mv.tile([128, 3, T], BF16, tag="sb")
                    nc.vector.tensor_tensor(
                        pm[:], dtsb[:, :, 0:3, :], dbb, mybir.AluOpType.mult
                    )
                    # balanced tree: sa = pm0+pm1 (Pool) || sb = pm2+S1 (DVE)
                    nc.gpsimd.tensor_tensor(
                        sa[:], pm[:, :, 0, :], pm[:, :, 1, :],
                        mybir.AluOpType.add,
                    )
                    nc.vector.tensor_tensor(
                        sb[:], pm[:, :, 2, :], s1p[:], mybir.AluOpType.add
                    )
                    nc.vector.tensor_tensor(
                        outs[:, nch], sa[:], sb[:], mybir.AluOpType.add
                    )
                    # out-DMA in 4-chunk groups, emitted 4 chunks late so
                    # the SP queue never parks on a long semaphore wait
                    if nb == 0 and nch == 0:
                        for later in range(1, NB):
                            emit_pft8(later)
                    if nch >= 7 and (nch - 7) % 4 == 0 and (
                        nch < 15 or nb == NB - 1
                    ):
                        g = (nch - 7) // 4
                        nc.sync.dma_start(
                            out_d[nb, :, 4 * g : 4 * g + 4],
                            outs[:, 4 * g : 4 * g + 4],
                        )
                prev_out = (nb, outs)
            # final two groups on different queues: their HWDGE holds overlap
            # (ACT's evac stream is finished by the time these fire)
            nc.sync.dma_start(out_d[NB - 1, :, 12:14], outs[:, 12:14])
            nc.scalar.dma_start(out_d[NB - 1, :, 14:16], outs[:, 14:16])

    nc.compile()
    return nc


def _prep_core(c, pose_body, trans, betas, A, v_template, shapedirs, posedirs,
               lbs_weights):
    bs = slice(NB * c, NB * (c + 1))
    pose = np.ascontiguousarray(
        pose_body[bs].transpose(1, 0, 2).reshape(T, NB, JB, 3)
    ).astype(np.float32)

    # pd8 [NB, 3, KDH, 2, N]: rows = 189 posedirs + 16 shapedirs + 1 pad,
    # split into the two DoubleRow K-groups, prescaled by PD_SCALE for fp8.
    pdc = posedirs[bs].reshape(NB, PF, N, 3).transpose(0, 3, 1, 2)  # [nb,c,p,n]
    sdc = shapedirs[bs].transpose(0, 2, 3, 1)                       # [nb,c,l,n]
    pcat = np.zeros((NB, 3, KD, N), dtype=np.float32)
    pcat[:, :, 0:PF] = pdc
    pcat[:, :, PF : PF + L] = sdc
    pd8 = np.ascontiguousarray(
        (pcat * PD_SCALE).reshape(NB, 3, 2, KDH, N).transpose(0, 1, 3, 2, 4)
    ).astype(NPF8)

    beta8 = np.ascontiguousarray(betas[bs].transpose(0, 2, 1)).astype(NPF8)

    w = lbs_weights[bs][:, :, 0:J]                                  # [nb, n, k]
    wt8 = np.ascontiguousarray(
        (w * PD_SCALE).transpose(0, 2, 1).reshape(NB, 2, KSH, N).transpose(0, 2, 1, 3)
    ).astype(NPF8)

    # ar8 [NB, KSH, 2, 3i, 3j, T] = A[g*KSH+k, i, j<3, t]
    akij = A[bs, :, :, 0:3, 0:3].transpose(0, 2, 3, 4, 1)           # [nb,k,i,j,t]
    ar8 = np.ascontiguousarray(
        akij.reshape(NB, 2, KSH, 3, 3, T).transpose(0, 2, 1, 3, 4, 5)
    ).astype(NPF8)

    # psit [NB, KT, N]: rows r=k*4+j -> w[n,k]*vth[n,j]; row 208 -> 1
    vth = np.concatenate(
        [v_template[bs], np.ones((NB, N, 1), dtype=np.float32)], axis=2
    )                                                               # [nb, n, 4]
    psi = (w[:, :, :, None] * vth[:, :, None, :]).reshape(NB, N, J * 4)
    psit = np.empty((NB, KT, N), dtype=NPBF16)
    psit[:, 0 : J * 4] = psi.transpose(0, 2, 1).astype(NPBF16)
    psit[:, J * 4] = np.ones((NB, N), dtype=NPBF16)

    # arb [NB, KT, 3, T]: rows r=k*4+j -> A[k,i,j,t]; row 208 -> trans[t,i]
    akji = A[bs, :, :, 0:3, :].transpose(0, 2, 4, 3, 1)             # [nb,k,j,i,t]
    arb = np.empty((NB, KT, 3, T), dtype=NPBF16)
    arb[:, 0 : J * 4] = akji.reshape(NB, J * 4, 3, T).astype(NPBF16)
    arb[:, J * 4] = trans[bs].transpose(0, 2, 1).astype(NPBF16)

    return {
        "pose": pose, "pd8": pd8, "beta8": beta8, "wt8": wt8, "ar8": ar8,
        "psit": psit, "arb": arb,
    }


def kernel(pose_body, trans, betas, A, v_template, shapedirs, posedirs,
           lbs_weights):
    if "nc" not in _CACHED:
        _CACHED["nc"] = _build_nc()
    nc = _CACHED["nc"]

    args = (pose_body, trans, betas, A, v_template, shapedirs, posedirs,
            lbs_weights)
    args = tuple(np.asarray(a, dtype=np.float32) for a in args)
    in_maps = [_prep_core(c, *args) for c in range(NCORES)]

    res = bass_utils.run_bass_kernel_spmd(nc, in_maps, core_ids=list(range(NCORES)))

    # out [NB, 128, NCH, 3, T] per core -> (B, T, N, 3); N = nch*128 + n128
    full = np.stack(
        [res.results[c]["out"].astype(np.float32) for c in range(NCORES)]
    )
    full = full.reshape(B, 128, NCH, 3, T).transpose(0, 4, 2, 1, 3)
    return np.ascontiguousarray(full.reshape(B, T, N, 3).astype(np.float32))


# revision 55
# speedup vs baseline: 1.0013x; 1.0013x over previous
"""Trainium2 Bass kernel for MinimalLBS (B=32, T=128, N=2048, J=52, Jb=21, L=16).

Strategy: data-parallel over B across 8 NeuronCores (4 samples per core).

Key algebraic restructure vs the naive path ("Psi-trick"):
  sens[n,i,t] = sum_{k,j} w[n,k] * A_aug[k,i,j,t] * vh[n,j,t]
with vh = vth + dh, vth = (v_template, 1), dh = (delta, 0),
delta = blend_shape + pose_offsets (small, ~0.05 sigma):

  S1[n,i,t] = sum_{(k,j)} Psi[n,(k,j)] * A_aug[(k,j),i,t]   (Psi = w (x) vth,
              t-independent -> host-computed, one bf16 matmul K=209; absorbs
              v_template, translation and the homogeneous j=3 column)
  S2[n,i,t] = sum_{j<3} ts[n,i,j,t] * delta[n,j,t]          (small correction;
              all inputs fp8 DoubleRow matmuls at 0.5 cycles/row)
  sens = S1 + S2

Per chunk of 128 vertices (per sample):
  PE : S1    = psit^T @ arb  2x bf16 matmuls (K=209) -> own 1-bank PSUM slot
       delta = pd8^T @ pft8  3x fp8-DR matmuls (K=206) -> spare tail row of
               each ts9 bank
       ts9   = wt8^T @ ar8   3x fp8-DR matmuls (K=52, j<3 only) -> 3-bank slot
  ACT: one copy evacuates delta+ts9 to bf16 SBUF; its scale slot undoes the
       x64 fp8 range prescale (this 1536-elem copy paces the pipeline)
  DVE: pm = ts9*delta (2x bf16), sb = pm2 + S1(PSUM), sens = sa + sb
  Pool: sa = pm0 + pm1
  Out-DMAs go in 4-chunk groups, emitted ~4 chunks late (and the last group
  after the next sample's input loads) so the SP queue never parks on a long
  semaphore wait and input prefetch is never blocked.
"""

import sys

sys.path.insert(0, "/opt/trn_rl_repo")

import math

import ml_dtypes
import numpy as np

import concourse.bacc as bacc
import concourse.bass as bass
import concourse.mybir as mybir
import concourse.tile as tile
from concourse import bass_utils, masks

F32 = mybir.dt.float32
BF16 = mybir.dt.bfloat16
F8 = mybir.dt.float8e4
NPBF16 = ml_dtypes.bfloat16
NPF8 = ml_dtypes.float8_e4m3fn
DR = mybir.MatmulPerfMode.DoubleRow

B, T, N, JB, J, L = 32, 128, 2048, 21, 52, 16
NCORES = 8
NB = B // NCORES          # samples per core
PF = JB * 9               # 189 pose-feature dims
NCH = N // 128            # n-chunks per sample
KD = PF + L + 1           # 206 logical K for the delta matmul (pad row last)
KDH = KD // 2             # 103
KS = J                    # 52 logical K for the ts9 matmul
KSH = KS // 2             # 26
KT = J * 4 + 1            # 209 logical K for the S1 matmul
KT0 = 128
KT1 = KT - 128            # 81
PD_SCALE = 64.0           # fp8 range prescale for posedirs/shapedirs

_CACHED = {}


def _build_nc():
    nc = bacc.Bacc("TRN2", target_bir_lowering=False, debug=False)

    pose_d = nc.dram_tensor("pose", [T, NB, JB, 3], F32, kind="ExternalInput")
    pd8_d = nc.dram_tensor("pd8", [NB, 3, KDH, 2, N], F8, kind="ExternalInput")
    beta8_d = nc.dram_tensor("beta8", [NB, L, T], F8, kind="ExternalInput")
    wt8_d = nc.dram_tensor("wt8", [NB, KSH, 2, N], F8, kind="ExternalInput")
    ar8_d = nc.dram_tensor("ar8", [NB, KSH, 2, 3, 3, T], F8, kind="ExternalInput")
    psit_d = nc.dram_tensor("psit", [NB, KT, N], BF16, kind="ExternalInput")
    arb_d = nc.dram_tensor("arb", [NB, KT, 3, T], BF16, kind="ExternalInput")
    out_d = nc.dram_tensor("out", [NB, 128, NCH, 3, T], BF16, kind="ExternalOutput")

    with tile.TileContext(nc) as tc:
        with (
            tc.tile_pool(name="const", bufs=1) as p_const,
            tc.tile_pool(name="rod", bufs=1) as p_rod,
            tc.tile_pool(name="big", bufs=2) as p_big,
            tc.tile_pool(name="small", bufs=2) as p_small,
            tc.tile_pool(name="mv", bufs=16) as p_mv,
            tc.tile_pool(name="ps", bufs=2, space="PSUM") as ps_ps,
            tc.tile_pool(name="s1p", bufs=2, space="PSUM") as ps_s1,
        ):
            ident = p_const.tile([128, 128], BF16)
            masks.make_identity(nc, ident[:])

            # ---- Rodrigues for all NB samples at once, ACT-free: with
            # y = |aa|^2, sinc(y) = sin(x)/x and g(y) = (1-cos x)/x^2 are
            # smooth in y, so (R - I) needs no sqrt/sin/reciprocal:
            #   diag_i   = g*(po_i^2 - y)
            #   offdiag  = g*po_a*po_b -+ sinc*po_c
            # Degree-3 series: error < 2e-5 over this pose range — far below
            # the fp8 quantization that follows.  Keeps the ACT table free
            # for the Copy-only steady state (no Sin/Sqrt table swaps).
            po = p_rod.tile([T, NB, JB, 3], F32)
            nc.sync.dma_start(po[:], pose_d[:])
            sq = p_rod.tile([T, NB, JB, 3], F32)
            nc.vector.tensor_tensor(sq[:], po[:], po[:], mybir.AluOpType.mult)
            a2 = p_rod.tile([T, NB, JB], F32)
            nc.vector.tensor_tensor(
                a2[:], sq[:, :, :, 0], sq[:, :, :, 1], mybir.AluOpType.add
            )
            y = p_rod.tile([T, NB, JB], F32)
            nc.vector.tensor_tensor(y[:], a2[:], sq[:, :, :, 2], mybir.AluOpType.add)

            def horner(coeffs, tag):
                # (((c3*y + c2)*y + c1)*y + c0  via alternating ts/tt ops
                h = p_rod.tile([T, NB, JB], F32, tag=f"{tag}_m")
                nc.vector.tensor_scalar(
                    h[:], y[:], coeffs[0], coeffs[1],
                    mybir.AluOpType.mult, mybir.AluOpType.add,
                )
                for ci, c in enumerate(coeffs[2:]):
                    hy = p_rod.tile([T, NB, JB], F32, tag=f"{tag}_h{ci}")
                    nc.vector.tensor_tensor(
                        hy[:], h[:], y[:], mybir.AluOpType.mult
                    )
                    h = p_rod.tile([T, NB, JB], F32, tag=f"{tag}_a{ci}")
                    nc.vector.tensor_scalar_add(h[:], hy[:], c)
                return h

            # 2/3-term series suffice: error ~5e-4 relative, far under the
            # fp8 quantization (1.7% rms) of the delta path these feed
            sinc = horner([1.0 / 120, -1.0 / 6, 1.0], "sinc")
            g = horner([1.0 / 720, -1.0 / 24, 0.5], "g")

            pf = p_rod.tile([T, NB, JB, 9], BF16)

            def poc(i):
                return po[:, :, :, i]

            prods = {}
            for (a, b2), nm in [((0, 1), "xy"), ((0, 2), "xz"), ((1, 2), "yz")]:
                t_ = p_rod.tile([T, NB, JB], F32, tag=f"pr_{nm}")
                nc.gpsimd.tensor_tensor(t_[:], poc(a), poc(b2), mybir.AluOpType.mult)
                prods[nm] = t_
            qs = {}
            for i, nm in [(0, "qx"), (1, "qy"), (2, "qz")]:
                t_ = p_rod.tile([T, NB, JB], F32, tag=f"q_{nm}")
                nc.vector.tensor_tensor(t_[:], sinc[:], poc(i), mybir.AluOpType.mult)
                qs[nm] = t_
            os_ = {}
            for nm in ["xy", "xz", "yz"]:
                t_ = p_rod.tile([T, NB, JB], F32, tag=f"o_{nm}")
                nc.vector.tensor_tensor(
                    t_[:], g[:], prods[nm][:], mybir.AluOpType.mult
                )
                os_[nm] = t_
            # diagonal: pf_di = g*(sq_i - y)
            d3 = p_rod.tile([T, NB, JB, 3], F32)
            nc.vector.tensor_tensor(
                d3[:], sq[:], y[:].unsqueeze(3).broadcast_to((T, NB, JB, 3)),
                mybir.AluOpType.subtract,
            )
            for di, i in [(0, 0), (4, 1), (8, 2)]:
                nc.vector.tensor_tensor(
                    pf[:, :, :, di], g[:], d3[:, :, :, i], mybir.AluOpType.mult
                )
            for ei, (e, o_nm, q_nm, op) in enumerate([
                (1, "xy", "qz", mybir.AluOpType.subtract),
                (3, "xy", "qz", mybir.AluOpType.add),
                (2, "xz", "qy", mybir.AluOpType.add),
                (6, "xz", "qy", mybir.AluOpType.subtract),
                (5, "yz", "qx", mybir.AluOpType.subtract),
                (7, "yz", "qx", mybir.AluOpType.add),
            ]):
                eng = nc.vector if ei % 2 == 0 else nc.gpsimd
                eng.tensor_tensor(
                    pf[:, :, :, e], os_[o_nm][:], qs[q_nm][:], op
                )

            def load_sample_inputs(nb, split=False):
                # For sample 0 the n-major tensors are split: the first 384
                # vertex columns (3 chunks' worth) land in ~1.5us so the
                # chunk pipeline starts while the bulk streams in behind.
                pd8_s = p_big.tile([KDH, 3, 2, N], F8, tag="pd8")
                pd8_src = pd8_d[nb].rearrange("c k g n -> k c g n")
                psit_a = p_small.tile([KT0, N], BF16, tag="psit_a")
                psit_b = p_small.tile([KT1, N], BF16, tag="psit_b")
                wt8_s = p_small.tile([KSH, 2, N], F8, tag="wt8")
                arb_a = p_small.tile([KT0, 3, T], BF16, tag="arb_a")
                arb_b = p_small.tile([KT1, 3, T], BF16, tag="arb_b")
                ar8_s = p_small.tile([KSH, 2, 3, 3, T], F8, tag="ar8")
                if split:
                    n1 = 384
                    for c in range(3):
                        nc.sync.dma_start(
                            pd8_s[:, c, :, 0:n1], pd8_src[:, c, :, 0:n1]
                        )
                else:
                    n1 = N
                    nc.sync.dma_start(pd8_s[:], pd8_src)
                nc.sync.dma_start(wt8_s[:, :, 0:n1], wt8_d[nb][:, :, 0:n1])
                nc.sync.dma_start(ar8_s[:], ar8_d[nb])
                nc.sync.dma_start(psit_a[:, 0:n1], psit_d[nb, 0:KT0, 0:n1])
                nc.sync.dma_start(arb_a[:], arb_d[nb, 0:KT0])
                nc.sync.dma_start(psit_b[:, 0:n1], psit_d[nb, KT0:KT, 0:n1])
                nc.sync.dma_start(arb_b[:], arb_d[nb, KT0:KT])
                if split:
                    for c in range(3):
                        nc.sync.dma_start(
                            pd8_s[:, c, :, n1:N], pd8_src[:, c, :, n1:N]
                        )
                    nc.sync.dma_start(psit_a[:, n1:N], psit_d[nb, 0:KT0, n1:N])
                    nc.sync.dma_start(wt8_s[:, :, n1:N], wt8_d[nb][:, :, n1:N])
                    nc.sync.dma_start(psit_b[:, n1:N], psit_d[nb, KT0:KT, n1:N])
                return pd8_s, wt8_s, ar8_s, psit_a, psit_b, arb_a, arb_b

            # beta loads ride the Pool software-DGE path (tiny transfers,
            # no HWDGE queue slot), then sample-0's bulk inputs
            pft8s = []
            for nb in range(NB):
                pft8 = p_const.tile([KDH, 2, T], F8, tag=f"pft8_{nb}")
                nc.vector.memset(pft8[:], 0.0)
                nc.gpsimd.dma_start(pft8[PF - KDH : PF - KDH + L, 1, :], beta8_d[nb])
                pft8s.append(pft8)
            s0_inputs = load_sample_inputs(0)

            # ---- pft8 transposes for ALL samples up front (PSUM slots and
            # PE are idle here; doing it per-sample stalled each boundary on
            # the s1p ring).  pft8 [103, 2, T]: fp8 DoubleRow K-groups of the
            # delta contraction: group0 = pf rows 0..102, group1 = pf rows
            # 103..188 ++ betas (16, DMA'd above) ++ zero pad row.  Sample
            # 0's copies run on ACT (ahead of the evac stream); later
            # samples' on DVE so they don't delay the first evacuations.
            # pack 4 transpose outputs (256B each) per 1-bank s1p slot so
            # the transposes never wait on the ring
            tp_a = ps_s1.tile([KDH, 8, T], BF16, tag="s1p")
            tp_b = ps_s1.tile([KDH, 8, T], BF16, tag="s1p")
            tps = [tp_a, tp_b]
            # dummy transposes into spare tp columns lift the PE out of its
            # cold p-state before the real transposes arrive
            for wi in (2, 3, 6):
                nc.tensor.transpose(
                    tp_a[:, wi, :], ident[:, 0:KDH], ident[:]
                )

            def emit_pft8(nb):
                pf_nb = pf[:, nb].rearrange("t j e -> t (j e)")
                tp = tps[nb // 2]
                o = 4 * (nb % 2)
                nc.tensor.transpose(tp[:, o, :], pf_nb[:, 0:KDH], ident[:])
                nc.tensor.transpose(
                    tp[0 : PF - KDH, o + 1, :], pf_nb[:, KDH:PF], ident[:]
                )
                pft8 = pft8s[nb]
                cp = nc.scalar.copy if nb == 0 else nc.vector.tensor_copy
                cp(pft8[:, 0, :], tp[:, o, :])
                cp(pft8[0 : PF - KDH, 1, :], tp[0 : PF - KDH, o + 1, :])

            # only sample 0's pft8 gates the first chunk; samples 1-3 are
            # emitted after chunk 0 so their transposes ride PE slack instead
            # of sitting ahead of chunk 0 in the PE stream
            emit_pft8(0)

            # ---- per-sample pipeline
            prev_out = None
            for nb in range(NB):
                pft8 = pft8s[nb]
                if nb == 0:
                    (pd8_s, wt8_s, ar8_s, psit_a, psit_b, arb_a,
                     arb_b) = s0_inputs
                else:
                    (pd8_s, wt8_s, ar8_s, psit_a, psit_b, arb_a,
                     arb_b) = load_sample_inputs(nb)

                # whole-sample output staging; DMA'd out in 4-chunk groups.
                # The previous sample's last half is emitted here, after this
                # sample's input loads, so its wait doesn't block them.
                if prev_out is not None:
                    nc.sync.dma_start(
                        out_d[prev_out[0], :, 8:16], prev_out[1][:, 8:16]
                    )
                outs = p_small.tile([128, NCH, 3, T], BF16, tag="outs")

                for nch in range(NCH):
                    n0 = nch * 128
                    # 3-bank slot: bank i holds ts9_i (rows 0:3) and delta_c=i
                    # in the spare tail row 3 — all matmul outs bank-local.
                    # delta/ts9 first in the PE stream: the evac only needs
                    # these, so a late psit/arb load can't stall it.
                    ps = ps_ps.tile([128, 3, 4, T], F32, tag="ps")
                    for c in range(3):
                        nc.tensor.matmul(
                            ps[:, c, 3, :],
                            pd8_s[:, c, :, n0 : n0 + 128],
                            pft8[:],
                            start=True, stop=True, perf_mode=DR,
                        )
                    for i in range(3):
                        nc.tensor.matmul(
                            ps[:, i, 0:3, :],
                            wt8_s[:, :, n0 : n0 + 128],
                            ar8_s[:, :, i],
                            start=True, stop=True, perf_mode=DR,
                        )
                    # S1 in its own 1-bank slot, only needed by sb later
                    s1p = ps_s1.tile([128, 3, T], F32, tag="s1p")
                    nc.tensor.matmul(
                        s1p[:], psit_a[:, n0 : n0 + 128], arb_a[:],
                        start=True, stop=False,
                    )
                    nc.tensor.matmul(
                        s1p[:], psit_b[:, n0 : n0 + 128], arb_b[:],
                        start=False, stop=True,
                    )
                    # single ACT copy evacuates delta + ts9 together; its
                    # scale slot undoes the x64 fp8 prescale on both factors
                    dtsb = p_mv.tile([128, 3, 4, T], BF16, tag="dtsb")
                    nc.scalar.mul(dtsb[:], ps[:], 1.0 / PD_SCALE)

                    dbb = (
                        dtsb[:, :, 3, :].unsqueeze(1)
                        .broadcast_to((128, 3, 3, T))
                    )
                    pm = p_mv.tile([128, 3, 3, T], BF16, tag="pm")
                    sa = p_mv.tile([128, 3, T], BF16, tag="sa")
                    sb = p_mv.tile([128, 3, T], BF16, tag="sb")
                    nc.vector.tensor_tensor(
                        pm[:], dtsb[:, :, 0:3, :], dbb, mybir.AluOpType.mult
                    )
                    # balanced tree: sa = pm0+pm1 (Pool) || sb = pm2+S1 (DVE)
                    nc.gpsimd.tensor_tensor(
                        sa[:], pm[:, :, 0, :], pm[:, :, 1, :],
                        mybir.AluOpType.add,
                    )
                    nc.vector.tensor_tensor(
                        sb[:], pm[:, :, 2, :], s1p[:], mybir.AluOpType.add
                    )
                    nc.vector.tensor_tensor(
                        outs[:, nch], sa[:], sb[:], mybir.AluOpType.add
                    )
                    # out-DMA in 4-chunk groups, emitted 4 chunks late so
                    # the SP queue never parks on a long semaphore wait
                    if nb == 0 and nch == 0:
                        for later in range(1, NB):
                            emit_pft8(later)
                    if nch >= 7 and (nch - 7) % 4 == 0 and (
                        nch < 15 or nb == NB - 1
                    ):
                        g = (nch - 7) // 4
                        nc.sync.dma_start(
                            out_d[nb, :, 4 * g : 4 * g + 4],
                            outs[:, 4 * g : 4 * g + 4],
                        )
                prev_out = (nb, outs)
            nc.sync.dma_start(out_d[NB - 1, :, 12:14], outs[:, 12:14])
            nc.sync.dma_start(out_d[NB - 1, :, 14:16], outs[:, 14:16])

    nc.compile()
    return nc


def _prep_core(c, pose_body, trans, betas, A, v_template, shapedirs, posedirs,
               lbs_weights):
    bs = slice(NB * c, NB * (c + 1))
    pose = np.ascontiguousarray(
        pose_body[bs].transpose(1, 0, 2).reshape(T, NB, JB, 3)
    ).astype(np.float32)

    # pd8 [NB, 3, KDH, 2, N]: rows = 189 posedirs + 16 shapedirs + 1 pad,
    # split into the two DoubleRow K-groups, prescaled by PD_SCALE for fp8.
    pdc = posedirs[bs].reshape(NB, PF, N, 3).transpose(0, 3, 1, 2)  # [nb,c,p,n]
    sdc = shapedirs[bs].transpose(0, 2, 3, 1)                       # [nb,c,l,n]
    pcat = np.zeros((NB, 3, KD, N), dtype=np.float32)
    pcat[:, :, 0:PF] = pdc
    pcat[:, :, PF : PF + L] = sdc
    pd8 = np.ascontiguousarray(
        (pcat * PD_SCALE).reshape(NB, 3, 2, KDH, N).transpose(0, 1, 3, 2, 4)
    ).astype(NPF8)

    beta8 = np.ascontiguousarray(betas[bs].transpose(0, 2, 1)).astype(NPF8)

    w = lbs_weights[bs][:, :, 0:J]                                  # [nb, n, k]
    wt8 = np.ascontiguousarray(
        (w * PD_SCALE).transpose(0, 2, 1).reshape(NB, 2, KSH, N).transpose(0, 2, 1, 3)
    ).astype(NPF8)

    # ar8 [NB, KSH, 2, 3i, 3j, T] = A[g*KSH+k, i, j<3, t]
    akij = A[bs, :, :, 0:3, 0:3].transpose(0, 2, 3, 4, 1)           # [nb,k,i,j,t]
    ar8 = np.ascontiguousarray(
        akij.reshape(NB, 2, KSH, 3, 3, T).transpose(0, 2, 1, 3, 4, 5)
    ).astype(NPF8)

    # psit [NB, KT, N]: rows r=k*4+j -> w[n,k]*vth[n,j]; row 208 -> 1
    vth = np.concatenate(
        [v_template[bs], np.ones((NB, N, 1), dtype=np.float32)], axis=2
    )                                                               # [nb, n, 4]
    psi = (w[:, :, :, None] * vth[:, :, None, :]).reshape(NB, N, J * 4)
    psit = np.empty((NB, KT, N), dtype=NPBF16)
    psit[:, 0 : J * 4] = psi.transpose(0, 2, 1).astype(NPBF16)
    psit[:, J * 4] = np.ones((NB, N), dtype=NPBF16)

    # arb [NB, KT, 3, T]: rows r=k*4+j -> A[k,i,j,t]; row 208 -> trans[t,i]
    akji = A[bs, :, :, 0:3, :].transpose(0, 2, 4, 3, 1)             # [nb,k,j,i,t]
    arb = np.empty((NB, KT, 3, T), dtype=NPBF16)
    arb[:, 0 : J * 4] = akji.reshape(NB, J * 4, 3, T).astype(NPBF16)
    arb[:, J * 4] = trans[bs].transpose(0, 2, 1).astype(NPBF16)

    return {
        "pose": pose, "pd8": pd8, "beta8": beta8, "wt8": wt8, "ar8": ar8,
        "psit": psit, "arb": arb,
    }


def kernel(pose_body, trans, betas, A, v_template, shapedirs, posedirs,
           lbs_weights):
    if "nc" not in _CACHED:
        _CACHED["nc"] = _build_nc()
    nc = _CACHED["nc"]

    args = (pose_body, trans, betas, A, v_template, shapedirs, posedirs,
            lbs_weights)
    args = tuple(np.asarray(a, dtype=np.float32) for a in args)
    in_maps = [_prep_core(c, *args) for c in range(NCORES)]

    res = bass_utils.run_bass_kernel_spmd(nc, in_maps, core_ids=list(range(NCORES)))

    # out [NB, 128, NCH, 3, T] per core -> (B, T, N, 3); N = nch*128 + n128
    full = np.stack(
        [res.results[c]["out"].astype(np.float32) for c in range(NCORES)]
    )
    full = full.reshape(B, 128, NCH, 3, T).transpose(0, 4, 2, 1, 3)
    return np.ascontiguousarray(full.reshape(B, T, N, 3).astype(np.float32))


# revision 56
# speedup vs baseline: 1.0024x; 1.0010x over previous
"""Trainium2 Bass kernel for MinimalLBS (B=32, T=128, N=2048, J=52, Jb=21, L=16).

Strategy: data-parallel over B across 8 NeuronCores (4 samples per core).

Key algebraic restructure vs the naive path ("Psi-trick"):
  sens[n,i,t] = sum_{k,j} w[n,k] * A_aug[k,i,j,t] * vh[n,j,t]
with vh = vth + dh, vth = (v_template, 1), dh = (delta, 0),
delta = blend_shape + pose_offsets (small, ~0.05 sigma):

  S1[n,i,t] = sum_{(k,j)} Psi[n,(k,j)] * A_aug[(k,j),i,t]   (Psi = w (x) vth,
              t-independent -> host-computed, one bf16 matmul K=209; absorbs
              v_template, translation and the homogeneous j=3 column)
  S2[n,i,t] = sum_{j<3} ts[n,i,j,t] * delta[n,j,t]          (small correction;
              all inputs fp8 DoubleRow matmuls at 0.5 cycles/row)
  sens = S1 + S2

Per chunk of 128 vertices (per sample):
  PE : S1    = psit^T @ arb  2x bf16 matmuls (K=209) -> own 1-bank PSUM slot
       delta = pd8^T @ pft8  3x fp8-DR matmuls (K=206) -> spare tail row of
               each ts9 bank
       ts9   = wt8^T @ ar8   3x fp8-DR matmuls (K=52, j<3 only) -> 3-bank slot
  ACT: one copy evacuates delta+ts9 to bf16 SBUF; its scale slot undoes the
       x64 fp8 range prescale (this 1536-elem copy paces the pipeline)
  DVE: pm = ts9*delta (2x bf16), sb = pm2 + S1(PSUM), sens = sa + sb
  Pool: sa = pm0 + pm1
  Out-DMAs go in 4-chunk groups, emitted ~4 chunks late (and the last group
  after the next sample's input loads) so the SP queue never parks on a long
  semaphore wait and input prefetch is never blocked.
"""

import sys

sys.path.insert(0, "/opt/trn_rl_repo")

import math

import ml_dtypes
import numpy as np

import concourse.bacc as bacc
import concourse.bass as bass
import concourse.mybir as mybir
import concourse.tile as tile
from concourse import bass_utils, masks

F32 = mybir.dt.float32
BF16 = mybir.dt.bfloat16
F8 = mybir.dt.float8e4
NPBF16 = ml_dtypes.bfloat16
NPF8 = ml_dtypes.float8_e4m3fn
DR = mybir.MatmulPerfMode.DoubleRow

B, T, N, JB, J, L = 32, 128, 2048, 21, 52, 16
NCORES = 8
NB = B // NCORES          # samples per core
PF = JB * 9               # 189 pose-feature dims
NCH = N // 128            # n-chunks per sample
KD = PF + L + 1           # 206 logical K for the delta matmul (pad row last)
KDH = KD // 2             # 103
KS = J                    # 52 logical K for the ts9 matmul
KSH = KS // 2             # 26
KT = J * 4 + 1            # 209 logical K for the S1 matmul
KT0 = 128
KT1 = KT - 128            # 81
PD_SCALE = 64.0           # fp8 range prescale for posedirs/shapedirs

_CACHED = {}


def _build_nc():
    nc = bacc.Bacc("TRN2", target_bir_lowering=False, debug=False)

    pose_d = nc.dram_tensor("pose", [T, NB, JB, 3], F32, kind="ExternalInput")
    pd8_d = nc.dram_tensor("pd8", [NB, 3, KDH, 2, N], F8, kind="ExternalInput")
    beta8_d = nc.dram_tensor("beta8", [NB, L, T], F8, kind="ExternalInput")
    wt8_d = nc.dram_tensor("wt8", [NB, KSH, 2, N], F8, kind="ExternalInput")
    ar8_d = nc.dram_tensor("ar8", [NB, KSH, 2, 3, 3, T], F8, kind="ExternalInput")
    psit_d = nc.dram_tensor("psit", [NB, KT, N], BF16, kind="ExternalInput")
    arb_d = nc.dram_tensor("arb", [NB, KT, 3, T], BF16, kind="ExternalInput")
    out_d = nc.dram_tensor("out", [NB, 128, NCH, 3, T], BF16, kind="ExternalOutput")

    with tile.TileContext(nc) as tc:
        with (
            tc.tile_pool(name="const", bufs=1) as p_const,
            tc.tile_pool(name="rod", bufs=1) as p_rod,
            tc.tile_pool(name="big", bufs=2) as p_big,
            tc.tile_pool(name="small", bufs=2) as p_small,
            tc.tile_pool(name="mv", bufs=16) as p_mv,
            tc.tile_pool(name="ps", bufs=2, space="PSUM") as ps_ps,
            tc.tile_pool(name="s1p", bufs=2, space="PSUM") as ps_s1,
        ):
            ident = p_const.tile([128, 128], BF16)
            masks.make_identity(nc, ident[:])

            # ---- Rodrigues for all NB samples at once, ACT-free: with
            # y = |aa|^2, sinc(y) = sin(x)/x and g(y) = (1-cos x)/x^2 are
            # smooth in y, so (R - I) needs no sqrt/sin/reciprocal:
            #   diag_i   = g*(po_i^2 - y)
            #   offdiag  = g*po_a*po_b -+ sinc*po_c
            # Degree-3 series: error < 2e-5 over this pose range — far below
            # the fp8 quantization that follows.  Keeps the ACT table free
            # for the Copy-only steady state (no Sin/Sqrt table swaps).
            po = p_rod.tile([T, NB, JB, 3], F32)
            nc.sync.dma_start(po[:], pose_d[:])
            sq = p_rod.tile([T, NB, JB, 3], F32)
            nc.vector.tensor_tensor(sq[:], po[:], po[:], mybir.AluOpType.mult)
            a2 = p_rod.tile([T, NB, JB], F32)
            nc.vector.tensor_tensor(
                a2[:], sq[:, :, :, 0], sq[:, :, :, 1], mybir.AluOpType.add
            )
            y = p_rod.tile([T, NB, JB], F32)
            nc.vector.tensor_tensor(y[:], a2[:], sq[:, :, :, 2], mybir.AluOpType.add)

            def horner(coeffs, tag):
                # (((c3*y + c2)*y + c1)*y + c0  via alternating ts/tt ops
                h = p_rod.tile([T, NB, JB], F32, tag=f"{tag}_m")
                nc.vector.tensor_scalar(
                    h[:], y[:], coeffs[0], coeffs[1],
                    mybir.AluOpType.mult, mybir.AluOpType.add,
                )
                for ci, c in enumerate(coeffs[2:]):
                    hy = p_rod.tile([T, NB, JB], F32, tag=f"{tag}_h{ci}")
                    nc.vector.tensor_tensor(
                        hy[:], h[:], y[:], mybir.AluOpType.mult
                    )
                    h = p_rod.tile([T, NB, JB], F32, tag=f"{tag}_a{ci}")
                    nc.vector.tensor_scalar_add(h[:], hy[:], c)
                return h

            # 2/3-term series suffice: error ~5e-4 relative, far under the
            # fp8 quantization (1.7% rms) of the delta path these feed
            sinc = horner([1.0 / 120, -1.0 / 6, 1.0], "sinc")
            g = horner([1.0 / 720, -1.0 / 24, 0.5], "g")

            pf = p_rod.tile([T, NB, JB, 9], BF16)

            def poc(i):
                return po[:, :, :, i]

            prods = {}
            for (a, b2), nm in [((0, 1), "xy"), ((0, 2), "xz"), ((1, 2), "yz")]:
                t_ = p_rod.tile([T, NB, JB], F32, tag=f"pr_{nm}")
                nc.gpsimd.tensor_tensor(t_[:], poc(a), poc(b2), mybir.AluOpType.mult)
                prods[nm] = t_
            qs = {}
            for i, nm in [(0, "qx"), (1, "qy"), (2, "qz")]:
                t_ = p_rod.tile([T, NB, JB], F32, tag=f"q_{nm}")
                nc.vector.tensor_tensor(t_[:], sinc[:], poc(i), mybir.AluOpType.mult)
                qs[nm] = t_
            os_ = {}
            for nm in ["xy", "xz", "yz"]:
                t_ = p_rod.tile([T, NB, JB], F32, tag=f"o_{nm}")
                nc.vector.tensor_tensor(
                    t_[:], g[:], prods[nm][:], mybir.AluOpType.mult
                )
                os_[nm] = t_
            # diagonal: pf_di = g*(sq_i - y)
            d3 = p_rod.tile([T, NB, JB, 3], F32)
            nc.vector.tensor_tensor(
                d3[:], sq[:], y[:].unsqueeze(3).broadcast_to((T, NB, JB, 3)),
                mybir.AluOpType.subtract,
            )
            for di, i in [(0, 0), (4, 1), (8, 2)]:
                nc.vector.tensor_tensor(
                    pf[:, :, :, di], g[:], d3[:, :, :, i], mybir.AluOpType.mult
                )
            for ei, (e, o_nm, q_nm, op) in enumerate([
                (1, "xy", "qz", mybir.AluOpType.subtract),
                (3, "xy", "qz", mybir.AluOpType.add),
                (2, "xz", "qy", mybir.AluOpType.add),
                (6, "xz", "qy", mybir.AluOpType.subtract),
                (5, "yz", "qx", mybir.AluOpType.subtract),
                (7, "yz", "qx", mybir.AluOpType.add),
            ]):
                eng = nc.vector if ei % 2 == 0 else nc.gpsimd
                eng.tensor_tensor(
                    pf[:, :, :, e], os_[o_nm][:], qs[q_nm][:], op
                )

            def load_sample_inputs(nb, split=False):
                # For sample 0 the n-major tensors are split: the first 384
                # vertex columns (3 chunks' worth) land in ~1.5us so the
                # chunk pipeline starts while the bulk streams in behind.
                pd8_s = p_big.tile([KDH, 3, 2, N], F8, tag="pd8")
                pd8_src = pd8_d[nb].rearrange("c k g n -> k c g n")
                psit_a = p_small.tile([KT0, N], BF16, tag="psit_a")
                psit_b = p_small.tile([KT1, N], BF16, tag="psit_b")
                wt8_s = p_small.tile([KSH, 2, N], F8, tag="wt8")
                arb_a = p_small.tile([KT0, 3, T], BF16, tag="arb_a")
                arb_b = p_small.tile([KT1, 3, T], BF16, tag="arb_b")
                ar8_s = p_small.tile([KSH, 2, 3, 3, T], F8, tag="ar8")
                if split:
                    n1 = 384
                    for c in range(3):
                        nc.sync.dma_start(
                            pd8_s[:, c, :, 0:n1], pd8_src[:, c, :, 0:n1]
                        )
                else:
                    n1 = N
                    nc.sync.dma_start(pd8_s[:], pd8_src)
                nc.sync.dma_start(wt8_s[:, :, 0:n1], wt8_d[nb][:, :, 0:n1])
                nc.sync.dma_start(ar8_s[:], ar8_d[nb])
                nc.sync.dma_start(psit_a[:, 0:n1], psit_d[nb, 0:KT0, 0:n1])
                nc.sync.dma_start(arb_a[:], arb_d[nb, 0:KT0])
                nc.sync.dma_start(psit_b[:, 0:n1], psit_d[nb, KT0:KT, 0:n1])
                nc.sync.dma_start(arb_b[:], arb_d[nb, KT0:KT])
                if split:
                    for c in range(3):
                        nc.sync.dma_start(
                            pd8_s[:, c, :, n1:N], pd8_src[:, c, :, n1:N]
                        )
                    nc.sync.dma_start(psit_a[:, n1:N], psit_d[nb, 0:KT0, n1:N])
                    nc.sync.dma_start(wt8_s[:, :, n1:N], wt8_d[nb][:, :, n1:N])
                    nc.sync.dma_start(psit_b[:, n1:N], psit_d[nb, KT0:KT, n1:N])
                return pd8_s, wt8_s, ar8_s, psit_a, psit_b, arb_a, arb_b

            # beta loads ride the Pool software-DGE path (tiny transfers,
            # no HWDGE queue slot), then sample-0's bulk inputs
            pft8s = []
            for nb in range(NB):
                pft8 = p_const.tile([KDH, 2, T], F8, tag=f"pft8_{nb}")
                nc.vector.memset(pft8[:], 0.0)
                nc.gpsimd.dma_start(pft8[PF - KDH : PF - KDH + L, 1, :], beta8_d[nb])
                pft8s.append(pft8)
            s0_inputs = load_sample_inputs(0)

            # ---- pft8 transposes for ALL samples up front (PSUM slots and
            # PE are idle here; doing it per-sample stalled each boundary on
            # the s1p ring).  pft8 [103, 2, T]: fp8 DoubleRow K-groups of the
            # delta contraction: group0 = pf rows 0..102, group1 = pf rows
            # 103..188 ++ betas (16, DMA'd above) ++ zero pad row.  Sample
            # 0's copies run on ACT (ahead of the evac stream); later
            # samples' on DVE so they don't delay the first evacuations.
            # pack 4 transpose outputs (256B each) per 1-bank s1p slot so
            # the transposes never wait on the ring
            tp_a = ps_s1.tile([KDH, 8, T], BF16, tag="s1p")
            tp_b = ps_s1.tile([KDH, 8, T], BF16, tag="s1p")
            tps = [tp_a, tp_b]
            # dummy transposes into spare tp columns lift the PE out of its
            # cold p-state just before the real transposes arrive: reading
            # the pf diagonal anchors them to ~the end of Rodrigues instead
            # of firing (and cooling again) at kernel start
            for wi, dc in ((2, 0), (3, 4), (6, 8)):
                nc.tensor.transpose(
                    tp_a[0:NB * JB, wi, :],
                    pf[:, :, :, dc].rearrange("t n j -> t (n j)"),
                    ident[:],
                )

            def emit_pft8(nb):
                pf_nb = pf[:, nb].rearrange("t j e -> t (j e)")
                tp = tps[nb // 2]
                o = 4 * (nb % 2)
                nc.tensor.transpose(tp[:, o, :], pf_nb[:, 0:KDH], ident[:])
                nc.tensor.transpose(
                    tp[0 : PF - KDH, o + 1, :], pf_nb[:, KDH:PF], ident[:]
                )
                pft8 = pft8s[nb]
                cp = nc.scalar.copy if nb == 0 else nc.vector.tensor_copy
                cp(pft8[:, 0, :], tp[:, o, :])
                cp(pft8[0 : PF - KDH, 1, :], tp[0 : PF - KDH, o + 1, :])

            # only sample 0's pft8 gates the first chunk; samples 1-3 are
            # emitted after chunk 0 so their transposes ride PE slack instead
            # of sitting ahead of chunk 0 in the PE stream
            emit_pft8(0)

            # ---- per-sample pipeline
            prev_out = None
            for nb in range(NB):
                pft8 = pft8s[nb]
                if nb == 0:
                    (pd8_s, wt8_s, ar8_s, psit_a, psit_b, arb_a,
                     arb_b) = s0_inputs
                else:
                    (pd8_s, wt8_s, ar8_s, psit_a, psit_b, arb_a,
                     arb_b) = load_sample_inputs(nb)

                # whole-sample output staging; DMA'd out in 4-chunk groups.
                # The previous sample's last half is emitted here, after this
                # sample's input loads, so its wait doesn't block them.
                if prev_out is not None:
                    nc.sync.dma_start(
                        out_d[prev_out[0], :, 8:16], prev_out[1][:, 8:16]
                    )
                outs = p_small.tile([128, NCH, 3, T], BF16, tag="outs")

                for nch in range(NCH):
                    n0 = nch * 128
                    # 3-bank slot: bank i holds ts9_i (rows 0:3) and delta_c=i
                    # in the spare tail row 3 — all matmul outs bank-local.
                    # delta/ts9 first in the PE stream: the evac only needs
                    # these, so a late psit/arb load can't stall it.
                    ps = ps_ps.tile([128, 3, 4, T], F32, tag="ps")
                    for c in range(3):
                        nc.tensor.matmul(
                            ps[:, c, 3, :],
                            pd8_s[:, c, :, n0 : n0 + 128],
                            pft8[:],
                            start=True, stop=True, perf_mode=DR,
                        )
                    for i in range(3):
                        nc.tensor.matmul(
                            ps[:, i, 0:3, :],
                            wt8_s[:, :, n0 : n0 + 128],
                            ar8_s[:, :, i],
                            start=True, stop=True, perf_mode=DR,
                        )
                    # S1 in its own 1-bank slot, only needed by sb later
                    s1p = ps_s1.tile([128, 3, T], F32, tag="s1p")
                    nc.tensor.matmul(
                        s1p[:], psit_a[:, n0 : n0 + 128], arb_a[:],
                        start=True, stop=False,
                    )
                    nc.tensor.matmul(
                        s1p[:], psit_b[:, n0 : n0 + 128], arb_b[:],
                        start=False, stop=True,
                    )
                    # single ACT copy evacuates delta + ts9 together; its
                    # scale slot undoes the x64 fp8 prescale on both factors
                    dtsb = p_mv.tile([128, 3, 4, T], BF16, tag="dtsb")
                    nc.scalar.mul(dtsb[:], ps[:], 1.0 / PD_SCALE)

                    dbb = (
                        dtsb[:, :, 3, :].unsqueeze(1)
                        .broadcast_to((128, 3, 3, T))
                    )
                    pm = p_mv.tile([128, 3, 3, T], BF16, tag="pm")
                    sa = p_mv.tile([128, 3, T], BF16, tag="sa")
                    sb = p_mv.tile([128, 3, T], BF16, tag="sb")
                    nc.vector.tensor_tensor(
                        pm[:], dtsb[:, :, 0:3, :], dbb, mybir.AluOpType.mult
                    )
                    # balanced tree: sa = pm0+pm1 (Pool) || sb = pm2+S1 (DVE)
                    nc.gpsimd.tensor_tensor(
                        sa[:], pm[:, :, 0, :], pm[:, :, 1, :],
                        mybir.AluOpType.add,
                    )
                    nc.vector.tensor_tensor(
                        sb[:], pm[:, :, 2, :], s1p[:], mybir.AluOpType.add
                    )
                    nc.vector.tensor_tensor(
                        outs[:, nch], sa[:], sb[:], mybir.AluOpType.add
                    )
                    # out-DMA in 4-chunk groups, emitted 4 chunks late so
                    # the SP queue never parks on a long semaphore wait
                    if nb == 0 and nch == 0:
                        for later in range(1, NB):
                            emit_pft8(later)
                    if nch >= 7 and (nch - 7) % 4 == 0 and (
                        nch < 15 or nb == NB - 1
                    ):
                        g = (nch - 7) // 4
                        nc.sync.dma_start(
                            out_d[nb, :, 4 * g : 4 * g + 4],
                            outs[:, 4 * g : 4 * g + 4],
                        )
                prev_out = (nb, outs)
            nc.sync.dma_start(out_d[NB - 1, :, 12:14], outs[:, 12:14])
            nc.sync.dma_start(out_d[NB - 1, :, 14:16], outs[:, 14:16])

    nc.compile()
    return nc


def _prep_core(c, pose_body, trans, betas, A, v_template, shapedirs, posedirs,
               lbs_weights):
    bs = slice(NB * c, NB * (c + 1))
    pose = np.ascontiguousarray(
        pose_body[bs].transpose(1, 0, 2).reshape(T, NB, JB, 3)
    ).astype(np.float32)

    # pd8 [NB, 3, KDH, 2, N]: rows = 189 posedirs + 16 shapedirs + 1 pad,
    # split into the two DoubleRow K-groups, prescaled by PD_SCALE for fp8.
    pdc = posedirs[bs].reshape(NB, PF, N, 3).transpose(0, 3, 1, 2)  # [nb,c,p,n]
    sdc = shapedirs[bs].transpose(0, 2, 3, 1)                       # [nb,c,l,n]
    pcat = np.zeros((NB, 3, KD, N), dtype=np.float32)
    pcat[:, :, 0:PF] = pdc
    pcat[:, :, PF : PF + L] = sdc
    pd8 = np.ascontiguousarray(
        (pcat * PD_SCALE).reshape(NB, 3, 2, KDH, N).transpose(0, 1, 3, 2, 4)
    ).astype(NPF8)

    beta8 = np.ascontiguousarray(betas[bs].transpose(0, 2, 1)).astype(NPF8)

    w = lbs_weights[bs][:, :, 0:J]                                  # [nb, n, k]
    wt8 = np.ascontiguousarray(
        (w * PD_SCALE).transpose(0, 2, 1).reshape(NB, 2, KSH, N).transpose(0, 2, 1, 3)
    ).astype(NPF8)

    # ar8 [NB, KSH, 2, 3i, 3j, T] = A[g*KSH+k, i, j<3, t]
    akij = A[bs, :, :, 0:3, 0:3].transpose(0, 2, 3, 4, 1)           # [nb,k,i,j,t]
    ar8 = np.ascontiguousarray(
        akij.reshape(NB, 2, KSH, 3, 3, T).transpose(0, 2, 1, 3, 4, 5)
    ).astype(NPF8)

    # psit [NB, KT, N]: rows r=k*4+j -> w[n,k]*vth[n,j]; row 208 -> 1
    vth = np.concatenate(
        [v_template[bs], np.ones((NB, N, 1), dtype=np.float32)], axis=2
    )                                                               # [nb, n, 4]
    psi = (w[:, :, :, None] * vth[:, :, None, :]).reshape(NB, N, J * 4)
    psit = np.empty((NB, KT, N), dtype=NPBF16)
    psit[:, 0 : J * 4] = psi.transpose(0, 2, 1).astype(NPBF16)
    psit[:, J * 4] = np.ones((NB, N), dtype=NPBF16)

    # arb [NB, KT, 3, T]: rows r=k*4+j -> A[k,i,j,t]; row 208 -> trans[t,i]
    akji = A[bs, :, :, 0:3, :].transpose(0, 2, 4, 3, 1)             # [nb,k,j,i,t]
    arb = np.empty((NB, KT, 3, T), dtype=NPBF16)
    arb[:, 0 : J * 4] = akji.reshape(NB, J * 4, 3, T).astype(NPBF16)
    arb[:, J * 4] = trans[bs].transpose(0, 2, 1).astype(NPBF16)

    return {
        "pose": pose, "pd8": pd8, "beta8": beta8, "wt8": wt8, "ar8": ar8,
        "psit": psit, "arb": arb,
    }


def kernel(pose_body, trans, betas, A, v_template, shapedirs, posedirs,
           lbs_weights):
    if "nc" not in _CACHED:
        _CACHED["nc"] = _build_nc()
    nc = _CACHED["nc"]

    args = (pose_body, trans, betas, A, v_template, shapedirs, posedirs,
            lbs_weights)
    args = tuple(np.asarray(a, dtype=np.float32) for a in args)
    in_maps = [_prep_core(c, *args) for c in range(NCORES)]

    res = bass_utils.run_bass_kernel_spmd(nc, in_maps, core_ids=list(range(NCORES)))

    # out [NB, 128, NCH, 3, T] per core -> (B, T, N, 3); N = nch*128 + n128
    full = np.stack(
        [res.results[c]["out"].astype(np.float32) for c in range(NCORES)]
    )
    full = full.reshape(B, 128, NCH, 3, T).transpose(0, 4, 2, 1, 3)
    return np.ascontiguousarray(full.reshape(B, T, N, 3).astype(np.float32))


# revision 57
# speedup vs baseline: 1.0037x; 1.0014x over previous
"""Trainium2 Bass kernel for MinimalLBS (B=32, T=128, N=2048, J=52, Jb=21, L=16).

Strategy: data-parallel over B across 8 NeuronCores (4 samples per core).

Key algebraic restructure vs the naive path ("Psi-trick"):
  sens[n,i,t] = sum_{k,j} w[n,k] * A_aug[k,i,j,t] * vh[n,j,t]
with vh = vth + dh, vth = (v_template, 1), dh = (delta, 0),
delta = blend_shape + pose_offsets (small, ~0.05 sigma):

  S1[n,i,t] = sum_{(k,j)} Psi[n,(k,j)] * A_aug[(k,j),i,t]   (Psi = w (x) vth,
              t-independent -> host-computed, one bf16 matmul K=209; absorbs
              v_template, translation and the homogeneous j=3 column)
  S2[n,i,t] = sum_{j<3} ts[n,i,j,t] * delta[n,j,t]          (small correction;
              all inputs fp8 DoubleRow matmuls at 0.5 cycles/row)
  sens = S1 + S2

Per chunk of 128 vertices (per sample):
  PE : S1    = psit^T @ arb  2x bf16 matmuls (K=209) -> own 1-bank PSUM slot
       delta = pd8^T @ pft8  3x fp8-DR matmuls (K=206) -> spare tail row of
               each ts9 bank
       ts9   = wt8^T @ ar8   3x fp8-DR matmuls (K=52, j<3 only) -> 3-bank slot
  ACT: one copy evacuates delta+ts9 to bf16 SBUF; its scale slot undoes the
       x64 fp8 range prescale (this 1536-elem copy paces the pipeline)
  DVE: pm = ts9*delta (2x bf16), sb = pm2 + S1(PSUM), sens = sa + sb
  Pool: sa = pm0 + pm1
  Out-DMAs go in 4-chunk groups, emitted ~4 chunks late (and the last group
  after the next sample's input loads) so the SP queue never parks on a long
  semaphore wait and input prefetch is never blocked.
"""

import sys

sys.path.insert(0, "/opt/trn_rl_repo")

import math

import ml_dtypes
import numpy as np

import concourse.bacc as bacc
import concourse.bass as bass
import concourse.mybir as mybir
import concourse.tile as tile
from concourse import bass_utils, masks

F32 = mybir.dt.float32
BF16 = mybir.dt.bfloat16
F8 = mybir.dt.float8e4
NPBF16 = ml_dtypes.bfloat16
NPF8 = ml_dtypes.float8_e4m3fn
DR = mybir.MatmulPerfMode.DoubleRow

B, T, N, JB, J, L = 32, 128, 2048, 21, 52, 16
NCORES = 8
NB = B // NCORES          # samples per core
PF = JB * 9               # 189 pose-feature dims
NCH = N // 128            # n-chunks per sample
KD = PF + L + 1           # 206 logical K for the delta matmul (pad row last)
KDH = KD // 2             # 103
KS = J                    # 52 logical K for the ts9 matmul
KSH = KS // 2             # 26
KT = J * 4 + 1            # 209 logical K for the S1 matmul
KT0 = 128
KT1 = KT - 128            # 81
PD_SCALE = 64.0           # fp8 range prescale for posedirs/shapedirs

_CACHED = {}


def _build_nc():
    nc = bacc.Bacc("TRN2", target_bir_lowering=False, debug=False)

    pose_d = nc.dram_tensor("pose", [T, NB, JB, 3], F32, kind="ExternalInput")
    pd8_d = nc.dram_tensor("pd8", [NB, 3, KDH, 2, N], F8, kind="ExternalInput")
    beta8_d = nc.dram_tensor("beta8", [NB, L, T], F8, kind="ExternalInput")
    wt8_d = nc.dram_tensor("wt8", [NB, KSH, 2, N], F8, kind="ExternalInput")
    ar8_d = nc.dram_tensor("ar8", [NB, KSH, 2, 3, 3, T], F8, kind="ExternalInput")
    psit_d = nc.dram_tensor("psit", [NB, KT, N], BF16, kind="ExternalInput")
    arb_d = nc.dram_tensor("arb", [NB, KT, 3, T], BF16, kind="ExternalInput")
    out_d = nc.dram_tensor("out", [NB, 128, NCH, 3, T], BF16, kind="ExternalOutput")

    with tile.TileContext(nc) as tc:
        with (
            tc.tile_pool(name="const", bufs=1) as p_const,
            tc.tile_pool(name="rod", bufs=1) as p_rod,
            tc.tile_pool(name="big", bufs=2) as p_big,
            tc.tile_pool(name="small", bufs=2) as p_small,
            tc.tile_pool(name="mv", bufs=16) as p_mv,
            tc.tile_pool(name="ps", bufs=2, space="PSUM") as ps_ps,
            tc.tile_pool(name="s1p", bufs=2, space="PSUM") as ps_s1,
        ):
            ident = p_const.tile([128, 128], BF16)
            masks.make_identity(nc, ident[:])

            # ---- Rodrigues for all NB samples at once, ACT-free: with
            # y = |aa|^2, sinc(y) = sin(x)/x and g(y) = (1-cos x)/x^2 are
            # smooth in y, so (R - I) needs no sqrt/sin/reciprocal:
            #   diag_i   = g*(po_i^2 - y)
            #   offdiag  = g*po_a*po_b -+ sinc*po_c
            # Degree-3 series: error < 2e-5 over this pose range — far below
            # the fp8 quantization that follows.  Keeps the ACT table free
            # for the Copy-only steady state (no Sin/Sqrt table swaps).
            po = p_rod.tile([T, NB, JB, 3], F32)
            nc.sync.dma_start(po[:], pose_d[:])
            sq = p_rod.tile([T, NB, JB, 3], F32)
            nc.vector.tensor_tensor(sq[:], po[:], po[:], mybir.AluOpType.mult)
            a2 = p_rod.tile([T, NB, JB], F32)
            nc.vector.tensor_tensor(
                a2[:], sq[:, :, :, 0], sq[:, :, :, 1], mybir.AluOpType.add
            )
            y = p_rod.tile([T, NB, JB], F32)
            nc.vector.tensor_tensor(y[:], a2[:], sq[:, :, :, 2], mybir.AluOpType.add)

            def horner(coeffs, tag):
                # (((c3*y + c2)*y + c1)*y + c0  via alternating ts/tt ops
                h = p_rod.tile([T, NB, JB], F32, tag=f"{tag}_m")
                nc.vector.tensor_scalar(
                    h[:], y[:], coeffs[0], coeffs[1],
                    mybir.AluOpType.mult, mybir.AluOpType.add,
                )
                for ci, c in enumerate(coeffs[2:]):
                    hy = p_rod.tile([T, NB, JB], F32, tag=f"{tag}_h{ci}")
                    nc.vector.tensor_tensor(
                        hy[:], h[:], y[:], mybir.AluOpType.mult
                    )
                    h = p_rod.tile([T, NB, JB], F32, tag=f"{tag}_a{ci}")
                    nc.vector.tensor_scalar_add(h[:], hy[:], c)
                return h

            # 2/3-term series suffice: error ~5e-4 relative, far under the
            # fp8 quantization (1.7% rms) of the delta path these feed
            sinc = horner([1.0 / 120, -1.0 / 6, 1.0], "sinc")
            g = horner([1.0 / 720, -1.0 / 24, 0.5], "g")

            pf = p_rod.tile([T, NB, JB, 9], BF16)

            def poc(i):
                return po[:, :, :, i]

            prods = {}
            for (a, b2), nm in [((0, 1), "xy"), ((0, 2), "xz"), ((1, 2), "yz")]:
                t_ = p_rod.tile([T, NB, JB], F32, tag=f"pr_{nm}")
                nc.gpsimd.tensor_tensor(t_[:], poc(a), poc(b2), mybir.AluOpType.mult)
                prods[nm] = t_
            qs = {}
            for i, nm in [(0, "qx"), (1, "qy"), (2, "qz")]:
                t_ = p_rod.tile([T, NB, JB], F32, tag=f"q_{nm}")
                nc.vector.tensor_tensor(t_[:], sinc[:], poc(i), mybir.AluOpType.mult)
                qs[nm] = t_
            os_ = {}
            for nm in ["xy", "xz", "yz"]:
                t_ = p_rod.tile([T, NB, JB], F32, tag=f"o_{nm}")
                nc.vector.tensor_tensor(
                    t_[:], g[:], prods[nm][:], mybir.AluOpType.mult
                )
                os_[nm] = t_
            # diagonal: pf_di = g*(sq_i - y)
            d3 = p_rod.tile([T, NB, JB, 3], F32)
            nc.vector.tensor_tensor(
                d3[:], sq[:], y[:].unsqueeze(3).broadcast_to((T, NB, JB, 3)),
                mybir.AluOpType.subtract,
            )
            for di, i in [(0, 0), (4, 1), (8, 2)]:
                nc.vector.tensor_tensor(
                    pf[:, :, :, di], g[:], d3[:, :, :, i], mybir.AluOpType.mult
                )
            for ei, (e, o_nm, q_nm, op) in enumerate([
                (1, "xy", "qz", mybir.AluOpType.subtract),
                (3, "xy", "qz", mybir.AluOpType.add),
                (2, "xz", "qy", mybir.AluOpType.add),
                (6, "xz", "qy", mybir.AluOpType.subtract),
                (5, "yz", "qx", mybir.AluOpType.subtract),
                (7, "yz", "qx", mybir.AluOpType.add),
            ]):
                eng = nc.vector if ei % 2 == 0 else nc.gpsimd
                eng.tensor_tensor(
                    pf[:, :, :, e], os_[o_nm][:], qs[q_nm][:], op
                )

            def load_sample_inputs(nb, split=False):
                # For sample 0 the n-major tensors are split: the first 384
                # vertex columns (3 chunks' worth) land in ~1.5us so the
                # chunk pipeline starts while the bulk streams in behind.
                pd8_s = p_big.tile([KDH, 3, 2, N], F8, tag="pd8")
                pd8_src = pd8_d[nb].rearrange("c k g n -> k c g n")
                psit_a = p_small.tile([KT0, N], BF16, tag="psit_a")
                psit_b = p_small.tile([KT1, N], BF16, tag="psit_b")
                wt8_s = p_small.tile([KSH, 2, N], F8, tag="wt8")
                arb_a = p_small.tile([KT0, 3, T], BF16, tag="arb_a")
                arb_b = p_small.tile([KT1, 3, T], BF16, tag="arb_b")
                ar8_s = p_small.tile([KSH, 2, 3, 3, T], F8, tag="ar8")
                if split:
                    n1 = 384
                    for c in range(3):
                        nc.sync.dma_start(
                            pd8_s[:, c, :, 0:n1], pd8_src[:, c, :, 0:n1]
                        )
                else:
                    n1 = N
                    nc.sync.dma_start(pd8_s[:], pd8_src)
                nc.sync.dma_start(wt8_s[:, :, 0:n1], wt8_d[nb][:, :, 0:n1])
                nc.sync.dma_start(ar8_s[:], ar8_d[nb])
                nc.sync.dma_start(psit_a[:, 0:n1], psit_d[nb, 0:KT0, 0:n1])
                nc.sync.dma_start(arb_a[:], arb_d[nb, 0:KT0])
                nc.sync.dma_start(psit_b[:, 0:n1], psit_d[nb, KT0:KT, 0:n1])
                nc.sync.dma_start(arb_b[:], arb_d[nb, KT0:KT])
                if split:
                    for c in range(3):
                        nc.sync.dma_start(
                            pd8_s[:, c, :, n1:N], pd8_src[:, c, :, n1:N]
                        )
                    nc.sync.dma_start(psit_a[:, n1:N], psit_d[nb, 0:KT0, n1:N])
                    nc.sync.dma_start(wt8_s[:, :, n1:N], wt8_d[nb][:, :, n1:N])
                    nc.sync.dma_start(psit_b[:, n1:N], psit_d[nb, KT0:KT, n1:N])
                return pd8_s, wt8_s, ar8_s, psit_a, psit_b, arb_a, arb_b

            # beta loads ride the Pool software-DGE path (tiny transfers,
            # no HWDGE queue slot), then sample-0's bulk inputs
            pft8s = []
            for nb in range(NB):
                pft8 = p_const.tile([KDH, 2, T], F8, tag=f"pft8_{nb}")
                nc.vector.memset(pft8[:], 0.0)
                nc.gpsimd.dma_start(pft8[PF - KDH : PF - KDH + L, 1, :], beta8_d[nb])
                pft8s.append(pft8)
            s0_inputs = load_sample_inputs(0)

            # ---- pft8 transposes for ALL samples up front (PSUM slots and
            # PE are idle here; doing it per-sample stalled each boundary on
            # the s1p ring).  pft8 [103, 2, T]: fp8 DoubleRow K-groups of the
            # delta contraction: group0 = pf rows 0..102, group1 = pf rows
            # 103..188 ++ betas (16, DMA'd above) ++ zero pad row.  Sample
            # 0's copies run on ACT (ahead of the evac stream); later
            # samples' on DVE so they don't delay the first evacuations.
            # pack 4 transpose outputs (256B each) per 1-bank s1p slot so
            # the transposes never wait on the ring
            tp_a = ps_s1.tile([KDH, 8, T], BF16, tag="s1p")
            tp_b = ps_s1.tile([KDH, 8, T], BF16, tag="s1p")
            tps = [tp_a, tp_b]
            # dummy transposes into spare tp columns lift the PE out of its
            # cold p-state just before the real transposes arrive: reading
            # the pf diagonal anchors them to ~the end of Rodrigues instead
            # of firing (and cooling again) at kernel start
            for wi, dc in ((2, 0), (3, 4), (6, 8)):
                nc.tensor.transpose(
                    tp_a[0:NB * JB, wi, :],
                    pf[:, :, :, dc].rearrange("t n j -> t (n j)"),
                    ident[:],
                )

            def emit_pft8(nb):
                pf_nb = pf[:, nb].rearrange("t j e -> t (j e)")
                tp = tps[nb // 2]
                o = 4 * (nb % 2)
                nc.tensor.transpose(tp[:, o, :], pf_nb[:, 0:KDH], ident[:])
                nc.tensor.transpose(
                    tp[0 : PF - KDH, o + 1, :], pf_nb[:, KDH:PF], ident[:]
                )
                pft8 = pft8s[nb]
                cp = nc.scalar.copy if nb == 0 else nc.vector.tensor_copy
                cp(pft8[:, 0, :], tp[:, o, :])
                cp(pft8[0 : PF - KDH, 1, :], tp[0 : PF - KDH, o + 1, :])

            # only sample 0's pft8 gates the first chunk; samples 1-3 are
            # emitted after chunk 0 so their transposes ride PE slack instead
            # of sitting ahead of chunk 0 in the PE stream
            emit_pft8(0)

            # ---- per-sample pipeline
            prev_out = None
            for nb in range(NB):
                pft8 = pft8s[nb]
                if nb == 0:
                    (pd8_s, wt8_s, ar8_s, psit_a, psit_b, arb_a,
                     arb_b) = s0_inputs
                else:
                    (pd8_s, wt8_s, ar8_s, psit_a, psit_b, arb_a,
                     arb_b) = load_sample_inputs(nb)

                # whole-sample output staging; DMA'd out in 4-chunk groups.
                # The previous sample's last half is emitted here, after this
                # sample's input loads, so its wait doesn't block them.
                if prev_out is not None:
                    nc.sync.dma_start(
                        out_d[prev_out[0], :, 12:16], prev_out[1][:, 12:16]
                    )
                outs = p_small.tile([128, NCH, 3, T], BF16, tag="outs")

                for nch in range(NCH):
                    n0 = nch * 128
                    # 3-bank slot: bank i holds ts9_i (rows 0:3) and delta_c=i
                    # in the spare tail row 3 — all matmul outs bank-local.
                    # delta/ts9 first in the PE stream: the evac only needs
                    # these, so a late psit/arb load can't stall it.
                    ps = ps_ps.tile([128, 3, 4, T], F32, tag="ps")
                    for c in range(3):
                        nc.tensor.matmul(
                            ps[:, c, 3, :],
                            pd8_s[:, c, :, n0 : n0 + 128],
                            pft8[:],
                            start=True, stop=True, perf_mode=DR,
                        )
                    for i in range(3):
                        nc.tensor.matmul(
                            ps[:, i, 0:3, :],
                            wt8_s[:, :, n0 : n0 + 128],
                            ar8_s[:, :, i],
                            start=True, stop=True, perf_mode=DR,
                        )
                    # S1 in its own 1-bank slot, only needed by sb later
                    s1p = ps_s1.tile([128, 3, T], F32, tag="s1p")
                    nc.tensor.matmul(
                        s1p[:], psit_a[:, n0 : n0 + 128], arb_a[:],
                        start=True, stop=False,
                    )
                    nc.tensor.matmul(
                        s1p[:], psit_b[:, n0 : n0 + 128], arb_b[:],
                        start=False, stop=True,
                    )
                    # single ACT copy evacuates delta + ts9 together; its
                    # scale slot undoes the x64 fp8 prescale on both factors
                    dtsb = p_mv.tile([128, 3, 4, T], BF16, tag="dtsb")
                    nc.scalar.mul(dtsb[:], ps[:], 1.0 / PD_SCALE)

                    dbb = (
                        dtsb[:, :, 3, :].unsqueeze(1)
                        .broadcast_to((128, 3, 3, T))
                    )
                    pm = p_mv.tile([128, 3, 3, T], BF16, tag="pm")
                    sa = p_mv.tile([128, 3, T], BF16, tag="sa")
                    sb = p_mv.tile([128, 3, T], BF16, tag="sb")
                    nc.vector.tensor_tensor(
                        pm[:], dtsb[:, :, 0:3, :], dbb, mybir.AluOpType.mult
                    )
                    # balanced tree: sa = pm0+pm1 (Pool) || sb = pm2+S1 (DVE)
                    nc.gpsimd.tensor_tensor(
                        sa[:], pm[:, :, 0, :], pm[:, :, 1, :],
                        mybir.AluOpType.add,
                    )
                    nc.vector.tensor_tensor(
                        sb[:], pm[:, :, 2, :], s1p[:], mybir.AluOpType.add
                    )
                    nc.vector.tensor_tensor(
                        outs[:, nch], sa[:], sb[:], mybir.AluOpType.add
                    )
                    # out-DMA in 4-chunk groups, emitted 4 chunks late so
                    # the SP queue never parks on a long semaphore wait
                    if nb == 0 and nch == 0:
                        for later in range(1, NB):
                            emit_pft8(later)
                    if nch >= 7 and (nch - 7) % 4 == 0:
                        g = (nch - 7) // 4
                        nc.sync.dma_start(
                            out_d[nb, :, 4 * g : 4 * g + 4],
                            outs[:, 4 * g : 4 * g + 4],
                        )
                prev_out = (nb, outs)
            nc.sync.dma_start(out_d[NB - 1, :, 12:14], outs[:, 12:14])
            nc.sync.dma_start(out_d[NB - 1, :, 14:16], outs[:, 14:16])

    nc.compile()
    return nc


def _prep_core(c, pose_body, trans, betas, A, v_template, shapedirs, posedirs,
               lbs_weights):
    bs = slice(NB * c, NB * (c + 1))
    pose = np.ascontiguousarray(
        pose_body[bs].transpose(1, 0, 2).reshape(T, NB, JB, 3)
    ).astype(np.float32)

    # pd8 [NB, 3, KDH, 2, N]: rows = 189 posedirs + 16 shapedirs + 1 pad,
    # split into the two DoubleRow K-groups, prescaled by PD_SCALE for fp8.
    pdc = posedirs[bs].reshape(NB, PF, N, 3).transpose(0, 3, 1, 2)  # [nb,c,p,n]
    sdc = shapedirs[bs].transpose(0, 2, 3, 1)                       # [nb,c,l,n]
    pcat = np.zeros((NB, 3, KD, N), dtype=np.float32)
    pcat[:, :, 0:PF] = pdc
    pcat[:, :, PF : PF + L] = sdc
    pd8 = np.ascontiguousarray(
        (pcat * PD_SCALE).reshape(NB, 3, 2, KDH, N).transpose(0, 1, 3, 2, 4)
    ).astype(NPF8)

    beta8 = np.ascontiguousarray(betas[bs].transpose(0, 2, 1)).astype(NPF8)

    w = lbs_weights[bs][:, :, 0:J]                                  # [nb, n, k]
    wt8 = np.ascontiguousarray(
        (w * PD_SCALE).transpose(0, 2, 1).reshape(NB, 2, KSH, N).transpose(0, 2, 1, 3)
    ).astype(NPF8)

    # ar8 [NB, KSH, 2, 3i, 3j, T] = A[g*KSH+k, i, j<3, t]
    akij = A[bs, :, :, 0:3, 0:3].transpose(0, 2, 3, 4, 1)           # [nb,k,i,j,t]
    ar8 = np.ascontiguousarray(
        akij.reshape(NB, 2, KSH, 3, 3, T).transpose(0, 2, 1, 3, 4, 5)
    ).astype(NPF8)

    # psit [NB, KT, N]: rows r=k*4+j -> w[n,k]*vth[n,j]; row 208 -> 1
    vth = np.concatenate(
        [v_template[bs], np.ones((NB, N, 1), dtype=np.float32)], axis=2
    )                                                               # [nb, n, 4]
    psi = (w[:, :, :, None] * vth[:, :, None, :]).reshape(NB, N, J * 4)
    psit = np.empty((NB, KT, N), dtype=NPBF16)
    psit[:, 0 : J * 4] = psi.transpose(0, 2, 1).astype(NPBF16)
    psit[:, J * 4] = np.ones((NB, N), dtype=NPBF16)

    # arb [NB, KT, 3, T]: rows r=k*4+j -> A[k,i,j,t]; row 208 -> trans[t,i]
    akji = A[bs, :, :, 0:3, :].transpose(0, 2, 4, 3, 1)             # [nb,k,j,i,t]
    arb = np.empty((NB, KT, 3, T), dtype=NPBF16)
    arb[:, 0 : J * 4] = akji.reshape(NB, J * 4, 3, T).astype(NPBF16)
    arb[:, J * 4] = trans[bs].transpose(0, 2, 1).astype(NPBF16)

    return {
        "pose": pose, "pd8": pd8, "beta8": beta8, "wt8": wt8, "ar8": ar8,
        "psit": psit, "arb": arb,
    }


def kernel(pose_body, trans, betas, A, v_template, shapedirs, posedirs,
           lbs_weights):
    if "nc" not in _CACHED:
        _CACHED["nc"] = _build_nc()
    nc = _CACHED["nc"]

    args = (pose_body, trans, betas, A, v_template, shapedirs, posedirs,
            lbs_weights)
    args = tuple(np.asarray(a, dtype=np.float32) for a in args)
    in_maps = [_prep_core(c, *args) for c in range(NCORES)]

    res = bass_utils.run_bass_kernel_spmd(nc, in_maps, core_ids=list(range(NCORES)))

    # out [NB, 128, NCH, 3, T] per core -> (B, T, N, 3); N = nch*128 + n128
    full = np.stack(
        [res.results[c]["out"].astype(np.float32) for c in range(NCORES)]
    )
    full = full.reshape(B, 128, NCH, 3, T).transpose(0, 4, 2, 1, 3)
    return np.ascontiguousarray(full.reshape(B, T, N, 3).astype(np.float32))
